# revision 35
# baseline (speedup 1.0000x reference)
"""CCRNN Trainium2 kernel: feature MLP + embedding lookup + 40-step LSTM + vocab projection.

Sharding: data-parallel over batch B=256 -> 8 cores x 32. Weights replicated.

v3 design (fp8-DoubleRow embedding-gate GEMM + select-seeded recurrence):
  ph1: gather shifted embeddings as fp8 rows (host pre-quantized emb8 = fp8(emb*64)),
       bf16-bounce PE-transpose -> embT fp8 [128, 4, 1280].
  ph2: features = X @ W_f.T + b_f -> featT; featT4 = featT col-replicated x4
       (token layout (t%4, b)); fgr [128, 8, 512] f32 = 4096*(feat@W_ih_feat.T
       + b_ih + b_hh) in token-replicated layout via M=128 GEMM.
  gx GEMM (chunks m=0..9 of 128 tokens; c0 pre-loop, rest streamed as quarter
       chunks 2/step into the loop's tail windows):
       pge [128 tok, 512] = embT-pairs x wihe-pairs (fp8 DoubleRow, 2 mms/chunk-n);
       ring[m][:, n, :] bf16 = pge + fgr[:, n, :]  (DVE add; ring pool bufs=6).
  loop t=0..39 (PSUM banks A/B [128(32q+b), 512]):
    seed: 4 col-tiled select-mms/bank (identb[:, 32*(t%4):+32] stationary) copy
    ring rows into psum (start=True); rec-mm (hsT x whh bf16, t>=1) accumulates;
    activations read PSUM with scale 2^-12; bank-B tail in (64, 192) slivers so
    hsT cols 0:64 land early and rec(t+1) w0..1 starts during the rest of the
    tail; one ph5 ch0 unit per step from t=16 fills the tail's PE window.
  ph5: logits bf16 [40vt, 3ch, 128, 512] = (psum * 2^-12 + b_out) (wout bf16
       streamed); 24 ch0 units run in-loop, the rest drain after with deep
       psum/output pipelining; host reorder.

Gate column permutation (newcol -> orig), n = a*4+q in 0..7, j in 0..511:
  a=0: [i_q | f_q], a=1: [g_q | o_q]  (torch gate order i,f,g,o; quarter q = H rows
  256q..256q+255).
Row (K-dim) permutation for hsT-layout: rp[128w+p] = 256*(p//32) + 32*w + (p%32).
"""

import os
import sys
import types
import contextlib

import numpy as np
import ml_dtypes

# ---- environment bootstrap (works in a bare dir; paths are machine-level) ----
for _p in ("/root/.axon_site", "/root/.axon_site/_ro/trn_rl_repo",
           "/root/.axon_site/_ro/pypackages"):
    if _p not in sys.path and os.path.isdir(_p):
        sys.path.insert(0, _p)

# shim the NTFF profile hook if the image's antenv lacks it (needed for trace=True)
try:
    from antenv.axon_hooks import get_axon_ntff_profile_hook  # noqa: F401
except ImportError:
    try:
        from trn_agent_boot.trn_boot import _ntff_profile_via_ctypes
        _m = types.ModuleType("antenv.axon_hooks")
        _hook = _ntff_profile_via_ctypes("/opt/axon/libaxon_pjrt.so")
        _m.get_axon_ntff_profile_hook = lambda: _hook
        _m.set_axon_ntff_profile_hook = lambda h: None
        import antenv  # noqa: F401
        sys.modules["antenv.axon_hooks"] = _m
    except Exception:
        pass

import concourse.bass as bass
import concourse.tile as tile
from concourse import mybir, bacc
from concourse import bass_utils
bass_utils.upload_artifacts = lambda tmpdir: f"file://{tmpdir}"
from concourse.bass import ds
from concourse.bass_utils import run_bass_kernel_spmd
from concourse.masks import make_identity

P = 128
B, T = 256, 40
IN, E, H, V = 2048, 512, 1024, 5000
G4 = 4 * H
NCORES = 8
BL = B // NCORES            # 32 per core
TB = T * BL                 # 1280 (t*32+b) rows
NM = TB // P                # 10 token chunks of 128
F32 = mybir.dt.float32
BF16 = mybir.dt.bfloat16
FP8 = mybir.dt.float8e4
I32 = mybir.dt.int32
DR = mybir.MatmulPerfMode.DoubleRow
Sig = mybir.ActivationFunctionType.Sigmoid
Tanh = mybir.ActivationFunctionType.Tanh
Copy = mybir.ActivationFunctionType.Copy
Ident = mybir.ActivationFunctionType.Identity

SGATE = 4096.0              # psum gate scale
ISG = 1.0 / SGATE
SE = 64.0                   # fp8 emb / wihe scale
VS = 40                     # vocab tiles of 128 (pad 5000 -> 5120)
VPAD = VS * P               # 5120
NCH = 3                     # (t,b) chunks: 512, 512, 256
RB = 6                      # gx ring buffers (chunks of 4 timesteps)


def gate_perm() -> np.ndarray:
    """newcol -> orig index over the 4096 gate dim."""
    perm = np.empty(G4, dtype=np.int64)
    for n in range(8):
        a, q = divmod(n, 4)
        for j in range(512):
            g = (0 if j < 256 else 1) + 2 * a          # i/f for bank0, g/o for bank1
            jj = j if j < 256 else j - 256
            perm[n * 512 + j] = g * H + q * 256 + jj
    return perm


def row_perm() -> np.ndarray:
    """rp[128w+p] = h-dim held at (partition p, k-tile w) in hsT layout."""
    rp = np.empty(H, dtype=np.int64)
    for w in range(8):
        for p in range(P):
            rp[w * P + p] = 256 * (p // 32) + 32 * w + (p % 32)
    return rp


def build_nc(debug: bool = False):
    nc = bacc.Bacc("TRN2", target_bir_lowering=False, debug=False)

    xt = nc.declare_dram_parameter("xt", [IN, BL], BF16, isOutput=False)
    lab = nc.declare_dram_parameter("lab", [TB, 1], I32, isOutput=False)
    wft = nc.declare_dram_parameter("wft", [IN, E], BF16, isOutput=False)
    bf = nc.declare_dram_parameter("bf", [1, E], F32, isOutput=False)
    emb8 = nc.declare_dram_parameter("emb8", [V + 1, E], FP8, isOutput=False)
    wihe = nc.declare_dram_parameter("wihe", [E, G4], FP8, isOutput=False)
    wihf = nc.declare_dram_parameter("wihf", [E + P, G4], BF16, isOutput=False)
    onepad = nc.declare_dram_parameter("onepad", [BL, P], F32, isOutput=False)
    whh = nc.declare_dram_parameter("whh", [H, G4], BF16, isOutput=False)
    wout = nc.declare_dram_parameter("wout", [H, VPAD], BF16, isOutput=False)
    boutT = nc.declare_dram_parameter("boutT", [P, VS], F32, isOutput=False)
    logits8 = nc.declare_dram_parameter("logits8", [VS, NCH, P, 512], BF16,
                                        isOutput=True)
    if debug:
        d_feat = nc.declare_dram_parameter("d_feat", [BL, E], F32, isOutput=True)
        d_embT = nc.declare_dram_parameter("d_embT", [P, 4, TB], FP8, isOutput=True)
        d_fgr = nc.declare_dram_parameter("d_fgr", [P, 8, 512], F32, isOutput=True)
        d_ring = nc.declare_dram_parameter("d_ring", [NM, P, 8, 512], BF16,
                                           isOutput=True)
        d_hs = nc.declare_dram_parameter("d_hs", [T, P, 256], BF16, isOutput=True)

    with tile.TileContext(nc) as tc, contextlib.ExitStack() as top:
        pc = top.enter_context(tc.tile_pool(name="pc", bufs=1))
        ident = pc.tile([P, P], F32, tag="ident")
        make_identity(nc, ident[:])
        identb = pc.tile([P, P], BF16, tag="identb")
        make_identity(nc, identb[:])

        # PE warmup during the initial DMA wait: junk matmuls (transpose-mode
        # does not engage HAM) ramp the clock gate to 8/8 before real work
        with tc.tile_pool(name="pwu", bufs=2, space="PSUM") as pwu:
            for _ in range(55):
                wut = pwu.tile([P, P], F32, space="PSUM", tag="wut")
                nc.tensor.matmul(wut[:], identb[:], identb[:], start=True,
                                 stop=True)

        # persistent SBUF residents
        prec = top.enter_context(tc.tile_pool(name="prec", bufs=1))
        whh_sb = prec.tile([P, 8, G4], BF16, tag="whh")
        wihe_sb = prec.tile([P, 4, G4], FP8, tag="wihe")
        embT = prec.tile([P, 4, TB], FP8, tag="embT")
        hsT = prec.tile([P, T + 1, 256], BF16, tag="hsT")
        fgr = prec.tile([P, 8, 512], F32, tag="fgr")
        cst = prec.tile([P, 256], F32, tag="cst")
        bouT_sb = prec.tile([P, VS], F32, tag="bouT")

        # ---------------- phase 1+2 DMAs, critical-path first ----------------
        with (
            tc.tile_pool(name="pgath", bufs=1) as pgath,
            tc.tile_pool(name="pgat2", bufs=3) as pgat2,
            tc.tile_pool(name="ps1", bufs=2, space="PSUM") as ps1,
            tc.tile_pool(name="p2", bufs=1) as p2,
            tc.tile_pool(name="p2w", bufs=4) as p2w,
            tc.tile_pool(name="p2wf", bufs=8) as p2wf,
            tc.tile_pool(name="ps2", bufs=2, space="PSUM") as ps2,
            tc.tile_pool(name="ps2f", bufs=2, space="PSUM") as ps2f,
        ):
            lab_sb = pgath.tile([P, NM, 1], I32, tag="lab")
            nc.sync.dma_start(lab_sb[:], lab.rearrange("(m p) o -> p m o", p=P))
            xt_sb = p2.tile([P, IN // P, BL], BF16, tag="xt")
            nc.sync.dma_start(xt_sb[:], xt.rearrange("(k p) b -> p k b", p=P))
            bf_sb = p2.tile([BL, E], F32, tag="bf")
            nc.sync.dma_start(bf_sb[:], bf[0][None, :].broadcast_to((BL, E)))
            one_sb = p2.tile([BL, P], F32, tag="one")
            nc.sync.dma_start(one_sb[:], onepad[:])
            wft3 = wft.rearrange("(k p) e -> p k e", p=P)
            wft_sbs = []
            for kc in range(4):
                wft_sb = p2w.tile([P, 4, E], BF16, tag="wft")
                nc.sync.dma_start(wft_sb[:], wft3[:, ds(kc * 4, 4), :])
                wft_sbs.append(wft_sb)
            wihf3 = wihf.rearrange("(k p) c -> p k c", p=P)
            wihf_sbs = []
            for n in range(8):
                wihf_sb = p2wf.tile([P, 5, 512], BF16, tag="wihf")
                nc.sync.dma_start(wihf_sb[:], wihf3[:, :, ds(n * 512, 512)])
                wihf_sbs.append(wihf_sb)
            wihe3 = wihe.rearrange("(k p) c -> p k c", p=P)
            for k in range(4):
                nc.sync.dma_start(wihe_sb[:, k, :], wihe3[:, k, :])
            whh3 = whh.rearrange("(k p) c -> p k c", p=P)
            for k in range(8):
                nc.sync.dma_start(whh_sb[:, k, :], whh3[:, k, :])
            nc.sync.dma_start(bouT_sb[:], boutT[:])

            # issue all gathers up-front (SWDGE, one persistent landing tile)
            fp = p2.tile([P, E + P], F32, tag="fp")
            nc.gpsimd.memset(fp[:], 0.0)
            gath8a = pgath.tile([P, NM, E], FP8, tag="gath8a")
            for m in range(NM):
                nc.gpsimd.indirect_dma_start(
                    out=gath8a[:, m, :], out_offset=None, in_=emb8[:],
                    in_offset=bass.IndirectOffsetOnAxis(ap=lab_sb[:, m, :], axis=0),
                )

            # features (emitted before embT work so each engine FIFO serves
            # the fgs-critical path first)
            psf = ps2f.tile([BL, E], F32, space="PSUM", tag="psf")
            nk = IN // P
            for kc in range(4):
                for k4 in range(4):
                    k = kc * 4 + k4
                    nc.tensor.matmul(psf[:], xt_sb[:, k, :], wft_sbs[kc][:, k4, :],
                                     start=(k == 0), stop=(k == nk - 1))
            nc.vector.tensor_add(fp[:BL, 0:E], psf[:], bf_sb[:])
            nc.vector.tensor_copy(fp[:BL, E:E + P], one_sb[:])
            if debug:
                nc.sync.dma_start(d_feat[:], fp[:BL, 0:E])

            # featT4: [E+pad chunk, kk, (tl, b)] -- batch cols replicated 4x
            featT4 = p2.tile([P, 5, P], BF16, tag="featT4")
            for kk in range(5):
                pst2 = ps2.tile([P, P], F32, space="PSUM", tag="pst2")
                nc.tensor.transpose(pst2[:], fp[:, ds(kk * P, P)], ident[:])
                for tl in range(4):
                    nc.vector.tensor_copy(featT4[:, kk, ds(32 * tl, 32)],
                                          pst2[:, 0:BL])

            # fgr GEMM: token-replicated feature gates (scaled by SGATE via wihf)
            for n in range(8):
                pfg = ps2f.tile([P, 512], F32, space="PSUM", tag="pfg")
                for kk in range(5):
                    nc.tensor.matmul(pfg[:], featT4[:, kk, :],
                                     wihf_sbs[n][:, kk, :],
                                     start=(kk == 0), stop=(kk == 4))
                nc.vector.tensor_copy(fgr[:, n, :], pfg[:])
            if debug:
                nc.sync.dma_start(d_fgr[:], fgr[:])

            # embT prep: fp8->bf16 convert + PE transpose + fp8 store
            for m in range(NM):
                gathb = pgat2.tile([P, E], BF16, tag="gathb")
                nc.scalar.copy(gathb[:], gath8a[:, m, :])
                for k in range(4):
                    pst = ps1.tile([P, P], BF16, space="PSUM", tag="pst")
                    nc.tensor.transpose(pst[:], gathb[:, ds(k * P, P)], identb[:])
                    if k % 2 == 0:
                        nc.vector.tensor_copy(embT[:, k, ds(m * P, P)], pst[:])
                    else:
                        nc.scalar.copy(embT[:, k, ds(m * P, P)], pst[:])
            if debug:
                nc.sync.dma_start(d_embT[:], embT[:])

        # ---------------- gx GEMM (fp8 DoubleRow) + recurrence + ph5 ----------------
        wout3 = wout.rearrange("(k p) v -> p k v", p=P)
        with (
            tc.tile_pool(name="psG", bufs=2, space="PSUM") as psG,
            tc.tile_pool(name="pring", bufs=RB) as pring,
            tc.tile_pool(name="psA", bufs=2, space="PSUM") as psA,
            tc.tile_pool(name="psB", bufs=2, space="PSUM") as psB,
            tc.tile_pool(name="p4", bufs=2) as p4,
            tc.tile_pool(name="ps5", bufs=2, space="PSUM") as ps5p,
            tc.tile_pool(name="p5o", bufs=3) as p5o,
            tc.tile_pool(name="p5w", bufs=2) as p5w,
        ):
            rings = {}
            wchs = {}

            def gx_quarter(m, nq):
                """ring[m][:, 2*nq:2*nq+2, :] = DR-GEMM(embT chunk m, wihe) + fgr."""
                if nq == 0:
                    ring_new = pring.tile([P, 8, 512], BF16, tag="ring")
                    rings[m] = ring_new
                ring_t = rings[m]
                pges = []
                for n2 in range(2):
                    pge = psG.tile([P, 512], F32, space="PSUM", tag="pge")
                    pges.append(pge)
                for w2 in range(2):
                    for n2 in range(2):
                        n = nq * 2 + n2
                        nc.tensor.matmul(
                            pges[n2][:],
                            embT[:, ds(2 * w2, 2), ds(m * P, P)],
                            wihe_sb[:, ds(2 * w2, 2), ds(n * 512, 512)],
                            start=(w2 == 0), stop=(w2 == 1), perf_mode=DR,
                            skip_group_check=True)
                for n2 in range(2):
                    n = nq * 2 + n2
                    nc.vector.tensor_add(ring_t[:, n, :], pges[n2][:],
                                         fgr[:, n, :])
                if debug and nq == 3:
                    nc.sync.dma_start(d_ring[m], ring_t[:])

            def gx_seed(t, pa, pb, stop=False):
                ring_t = rings[t // 4]
                tl = t % 4
                for a, ps_ in ((0, pa), (1, pb)):
                    for q in range(4):
                        nc.tensor.matmul(
                            ps_[ds(32 * q, 32), :], identb[:, ds(32 * tl, 32)],
                            ring_t[:, 4 * a + q, :], start=True, stop=stop,
                            tile_position=(0, 32 * q), skip_group_check=True)

            def rec_bank(t, a, ps_):
                for w in range(8):
                    for q in range(4):
                        nc.tensor.matmul(
                            ps_[ds(32 * q, 32), :],
                            hsT[:, t, ds(32 * w, 32)],
                            whh_sb[:, w, ds((4 * a + q) * 512, 512)],
                            start=False, stop=(w == 7), tile_position=(0, 32 * q),
                            skip_group_check=True)

            def ph5_unit(vs, ch, pools=None):
                """One [128 vocab x N tb] output tile: 8 mms + bias act + DMA."""
                psum_p, out_p = pools or (ps5p, p5o)
                vc, vl = divmod(vs, 4)
                N = 512 if ch < 2 else 256
                pu = psum_p.tile([P, 512], F32, space="PSUM", tag="pu")
                for w in range(8):
                    nc.tensor.matmul(
                        pu[:, 0:N],
                        wchs[vc][:, w, ds(128 * vl, P)],
                        hsT[:, ds(1 + 16 * ch, N // 32), ds(32 * w, 32)],
                        start=(w == 0), stop=(w == 7))
                osb = out_p.tile([P, 512], BF16, tag="osb")
                nc.scalar.activation(osb[:, 0:N], pu[:, 0:N], Ident,
                                     bias=bouT_sb[:, ds(vs, 1)])
                nc.sync.dma_start(logits8[vs, ch, :, 0:N], osb[:, 0:N])

            def wch_dma(vc, pool=None):
                wch = (pool or p5w).tile([P, 8, 512], BF16, tag="wch")
                wchs[vc] = wch
                nc.sync.dma_start(wch[:], wout3[:, :, ds(512 * vc, 512)])

            nc.gpsimd.memset(cst[:], 0.0)

            # preload ring chunk 0; chunks 1..9 stream as quarters in-loop
            for nq in range(4):
                gx_quarter(0, nq)
            wch_dma(0)

            pa = psA.tile([P, 512], F32, space="PSUM", tag="pa")
            pb = psB.tile([P, 512], F32, space="PSUM", tag="pb")
            gx_seed(0, pa, pb, stop=True)

            # sliver layout for the bank-B tail
            SLV = ((0, 64), (64, 192))
            for t in range(T):
                # ph5 ch0 units from t=16 (hsT slots 1..16 ready), one per step
                if t >= 16 and (t - 16) < 24:
                    vs = t - 16
                    if vs % 4 == 0 and vs // 4 + 1 < 6:
                        wch_dma(vs // 4 + 1)
                    ph5_unit(vs, 0)
                if t > 0:
                    rec_bank(t, 0, pa)
                # bank A activations: sif = sig([i|f]); csf = sf * c
                sif = p4.tile([P, 512], BF16, tag="sif")
                csf = p4.tile([P, 256], F32, tag="csf")
                nc.scalar.activation(sif[:], pa[:], Sig, scale=ISG)
                nc.gpsimd.tensor_mul(csf[:], sif[:, 256:512], cst[:])

                if t > 0:
                    rec_bank(t, 1, pb)
                # seeds for next step fill the PE while the B-side tail runs
                if t + 1 < T:
                    pa2 = psA.tile([P, 512], F32, space="PSUM", tag="pa")
                    pb2 = psB.tile([P, 512], F32, space="PSUM", tag="pb")
                    gx_seed(t + 1, pa2, pb2)
                # bank B tail in slivers: tg = tanh(g); t3 = si*tg; c = csf+t3;
                # tcc = tanh(c); h = so*tcc; hsT[t+1] sliver = blockT(h sliver)
                tg = p4.tile([P, 256], BF16, tag="tg")
                so = p4.tile([P, 256], BF16, tag="so")
                tcc = p4.tile([P, 256], BF16, tag="tcc")
                t3 = p4.tile([P, 256], F32, tag="t3")
                h128 = p4.tile([P, 256], BF16, tag="h128")
                for (o, w_) in SLV:
                    nc.scalar.activation(tg[:, ds(o, w_)], pb[:, ds(o, w_)],
                                         Tanh, scale=ISG)
                nc.scalar.activation(so[:], pb[:, 256:512], Sig, scale=ISG)
                for (o, w_) in SLV:
                    sl = ds(o, w_)
                    nc.vector.tensor_mul(t3[:, sl], sif[:, sl], tg[:, sl])
                    nc.vector.tensor_add(cst[:, sl], csf[:, sl], t3[:, sl])
                    nc.scalar.activation(tcc[:, sl], cst[:, sl], Tanh)
                    nc.vector.tensor_mul(h128[:, sl], so[:, sl], tcc[:, sl])
                    nc.vector.transpose(hsT[:, t + 1, sl], h128[:, sl])
                if debug:
                    nc.sync.dma_start(d_hs[t], h128[:])
                # stream gx quarters, 2/step: chunk 1+t//2 over t=0..15, c9 at 16-17
                if t < 16:
                    m2 = 1 + t // 2
                    gx_quarter(m2, 2 * (t % 2))
                    gx_quarter(m2, 2 * (t % 2) + 1)
                elif t < 18:
                    gx_quarter(9, 2 * (t - 16))
                    gx_quarter(9, 2 * (t - 16) + 1)
                if t + 1 < T:
                    pa, pb = pa2, pb2

        # ---------------- ph5 drain (remaining units, deep pipelining) ----------------
        with (
            tc.tile_pool(name="ps5d", bufs=4, space="PSUM") as ps5d,
            tc.tile_pool(name="p5od", bufs=8) as p5od,
            tc.tile_pool(name="p5wd", bufs=3) as p5wd,
        ):
            for vc in range(VPAD // 512):
                wch_dma(vc, pool=p5wd)
                for vl in range(4):
                    vs = 4 * vc + vl
                    for ch in range(NCH):
                        if ch == 0 and vs < VS - 16:
                            continue  # done in-loop
                        ph5_unit(vs, ch, pools=(ps5d, p5od))

    nc.finalize()
    return nc


_NC_CACHE: dict = {}


def _get_nc(debug: bool = False):
    key = bool(debug)
    if key not in _NC_CACHE:
        _NC_CACHE[key] = build_nc(debug=key)
    return _NC_CACHE[key]


def host_prep(inputs: dict) -> list[dict]:
    """Shard + lay out inputs for the 8 cores."""
    X = np.asarray(inputs["X"], dtype=np.float32)
    labels = np.asarray(inputs["labels"])
    W_f = np.asarray(inputs["W_f"], dtype=np.float32)
    b_f = np.asarray(inputs["b_f"], dtype=np.float32)
    emb = np.asarray(inputs["emb"], dtype=np.float32)
    W_ih = np.asarray(inputs["W_ih"], dtype=np.float32)
    W_hh = np.asarray(inputs["W_hh"], dtype=np.float32)
    b_ih = np.asarray(inputs["b_ih"], dtype=np.float32)
    b_hh = np.asarray(inputs["b_hh"], dtype=np.float32)
    W_out = np.asarray(inputs["W_out"], dtype=np.float32)
    b_out = np.asarray(inputs["b_out"], dtype=np.float32)

    perm = gate_perm()
    rp = row_perm()
    bff = ml_dtypes.bfloat16
    f8 = ml_dtypes.float8_e4m3fn
    wft = np.ascontiguousarray(W_f.T).astype(bff)                      # [IN, E]
    emb8 = np.clip(emb * SE, -240.0, 240.0).astype(f8)                 # [V+1, E]
    wihe = np.ascontiguousarray(W_ih[:, E:].T[:, perm] * SE).astype(f8)
    wihf_aug = np.zeros((E + P, G4), dtype=np.float32)
    wihf_aug[:E] = W_ih[:, :E].T[:, perm] * SGATE
    wihf_aug[E] = (b_ih + b_hh)[perm] * SGATE
    wihf_aug = wihf_aug.astype(bff)
    whh = np.ascontiguousarray((W_hh.T * SGATE)[rp][:, perm]).astype(bff)
    wout_p = np.zeros((H, VPAD), dtype=np.float32)
    wout_p[:, :V] = W_out.T[rp]
    wout_p = wout_p.astype(bff)
    boutT = np.zeros((P, VS), dtype=np.float32)
    boutT.T.reshape(-1)[:V] = b_out
    onepad = np.zeros((BL, P), dtype=np.float32)
    onepad[:, 0] = 1.0

    shared = {
        "wft": wft, "bf": b_f[None, :], "emb8": emb8, "wihe": wihe,
        "wihf": wihf_aug, "onepad": onepad, "whh": whh, "wout": wout_p,
        "boutT": boutT,
    }
    shifted = np.roll(labels, 1, axis=1)                               # [B, T]
    in_maps = []
    for c in range(NCORES):
        s = slice(c * BL, (c + 1) * BL)
        xt = np.ascontiguousarray(X[s].T).astype(bff)                  # [IN, 32]
        lab = np.ascontiguousarray(shifted[s].T.reshape(TB, 1)).astype(np.int32)
        in_maps.append({**shared, "xt": xt, "lab": lab})
    return in_maps


def unpack_logits(raw: np.ndarray) -> np.ndarray:
    """[VS, NCH, 128, 512] bf16 -> [BL, T, V] fp32."""
    arr = np.asarray(raw).astype(np.float32)         # [40, 3, 128, 512]
    flat = arr.transpose(1, 3, 0, 2).reshape(NCH * 512, VPAD)  # [tb', v]
    flat = flat[:TB, :V]                             # [1280, 5000]
    return np.ascontiguousarray(
        flat.reshape(T, BL, V).transpose(1, 0, 2))   # [32, 40, 5000]


def run(inputs: dict, debug: bool = False, trace: bool = False):
    nc = _get_nc(debug=debug)
    in_maps = host_prep(inputs)
    r = run_bass_kernel_spmd(nc, in_maps, core_ids=list(range(NCORES)), trace=trace)
    outs = [unpack_logits(r.results[c]["logits8"]) for c in range(NCORES)]
    out = np.concatenate(outs, axis=0)
    return out, r


def kernel(**inputs) -> np.ndarray:
    out, _ = run(inputs, debug=False, trace=False)
    return out


if __name__ == "__main__":
    rng = np.random.default_rng(0)
    fake = {
        "X": rng.standard_normal((B, IN)).astype(np.float32),
        "labels": rng.integers(0, V, size=(B, T)).astype(np.int64),
        "W_f": (rng.standard_normal((E, IN)) * 0.02).astype(np.float32),
        "b_f": np.zeros(E, np.float32),
        "emb": (rng.standard_normal((V + 1, E)) * 0.02).astype(np.float32),
        "W_ih": (rng.standard_normal((G4, 2 * E)) * 0.02).astype(np.float32),
        "W_hh": (rng.standard_normal((G4, H)) * 0.02).astype(np.float32),
        "b_ih": np.zeros(G4, np.float32),
        "b_hh": np.zeros(G4, np.float32),
        "W_out": (rng.standard_normal((V, H)) * 0.02).astype(np.float32),
        "b_out": np.zeros(V, np.float32),
    }
    out = kernel(**fake)
    print("out", out.shape, out.dtype, float(np.abs(out).max()))


# revision 36
# speedup vs baseline: 1.1690x; 1.1690x over previous
"""CCRNN Trainium2 kernel: feature MLP + embedding lookup + 40-step LSTM + vocab projection.

Sharding: data-parallel over batch B=256 -> 8 cores x 32. Weights replicated.

v3 design (fp8-DoubleRow embedding-gate GEMM + select-seeded recurrence):
  ph1: gather shifted embeddings as fp8 rows (host pre-quantized emb8 = fp8(emb*64)),
       bf16-bounce PE-transpose -> embT fp8 [128, 4, 1280].
  ph2: features = X @ W_f.T + b_f -> featT; featT4 = featT col-replicated x4
       (token layout (t%4, b)); fgr [128, 8, 512] f32 = 4096*(feat@W_ih_feat.T
       + b_ih + b_hh) in token-replicated layout via M=128 GEMM.
  gx GEMM (chunks m=0..9 of 128 tokens; c0 pre-loop, rest streamed as quarter
       chunks 2/step into the loop's tail windows):
       pge [128 tok, 512] = embT-pairs x wihe-pairs (fp8 DoubleRow, 2 mms/chunk-n);
       ring[m][:, n, :] bf16 = pge + fgr[:, n, :]  (DVE add; ring pool bufs=6).
  loop t=0..39 (PSUM banks A/B [128(32q+b), 512]):
    seed: 4 col-tiled select-mms/bank (identb[:, 32*(t%4):+32] stationary) copy
    ring rows into psum (start=True); rec-mm (hsT x whh bf16, t>=1) accumulates;
    activations read PSUM with scale 2^-12; bank-B tail in (64, 192) slivers so
    hsT cols 0:64 land early and rec(t+1) w0..1 starts during the rest of the
    tail; one ph5 ch0 unit per step from t=16 fills the tail's PE window.
  ph5: logits bf16 [40vt, 3ch, 128, 512] = (psum * 2^-12 + b_out) (wout bf16
       streamed); 24 ch0 units run in-loop, the rest drain after with deep
       psum/output pipelining; host reorder.

Gate column permutation (newcol -> orig), n = a*4+q in 0..7, j in 0..511:
  a=0: [i_q | f_q], a=1: [g_q | o_q]  (torch gate order i,f,g,o; quarter q = H rows
  256q..256q+255).
Row (K-dim) permutation for hsT-layout: rp[128w+p] = 256*(p//32) + 32*w + (p%32).
"""

import os
import sys
import types
import contextlib

import numpy as np
import ml_dtypes

# ---- environment bootstrap (works in a bare dir; paths are machine-level) ----
for _p in ("/root/.axon_site", "/root/.axon_site/_ro/trn_rl_repo",
           "/root/.axon_site/_ro/pypackages"):
    if _p not in sys.path and os.path.isdir(_p):
        sys.path.insert(0, _p)

# shim the NTFF profile hook if the image's antenv lacks it (needed for trace=True)
try:
    from antenv.axon_hooks import get_axon_ntff_profile_hook  # noqa: F401
except ImportError:
    try:
        from trn_agent_boot.trn_boot import _ntff_profile_via_ctypes
        _m = types.ModuleType("antenv.axon_hooks")
        _hook = _ntff_profile_via_ctypes("/opt/axon/libaxon_pjrt.so")
        _m.get_axon_ntff_profile_hook = lambda: _hook
        _m.set_axon_ntff_profile_hook = lambda h: None
        import antenv  # noqa: F401
        sys.modules["antenv.axon_hooks"] = _m
    except Exception:
        pass

import concourse.bass as bass
import concourse.tile as tile
from concourse import mybir, bacc
from concourse import bass_utils
bass_utils.upload_artifacts = lambda tmpdir: f"file://{tmpdir}"
from concourse.bass import ds
from concourse.bass_utils import run_bass_kernel_spmd
from concourse.masks import make_identity

P = 128
B, T = 256, 40
IN, E, H, V = 2048, 512, 1024, 5000
G4 = 4 * H
NCORES = 8
BL = B // NCORES            # 32 per core
TB = T * BL                 # 1280 (t*32+b) rows
NM = TB // P                # 10 token chunks of 128
F32 = mybir.dt.float32
BF16 = mybir.dt.bfloat16
FP8 = mybir.dt.float8e4
I32 = mybir.dt.int32
DR = mybir.MatmulPerfMode.DoubleRow
Sig = mybir.ActivationFunctionType.Sigmoid
Tanh = mybir.ActivationFunctionType.Tanh
Copy = mybir.ActivationFunctionType.Copy
Ident = mybir.ActivationFunctionType.Identity

SGATE = 4096.0              # psum gate scale
ISG = 1.0 / SGATE
SE = 64.0                   # fp8 emb / wihe scale
VS = 40                     # vocab tiles of 128 (pad 5000 -> 5120)
VPAD = VS * P               # 5120
NCH = 3                     # (t,b) chunks: 512, 512, 256
RB = 6                      # gx ring buffers (chunks of 4 timesteps)


def gate_perm() -> np.ndarray:
    """newcol -> orig index over the 4096 gate dim."""
    perm = np.empty(G4, dtype=np.int64)
    for n in range(8):
        a, q = divmod(n, 4)
        for j in range(512):
            g = (0 if j < 256 else 1) + 2 * a          # i/f for bank0, g/o for bank1
            jj = j if j < 256 else j - 256
            perm[n * 512 + j] = g * H + q * 256 + jj
    return perm


def row_perm() -> np.ndarray:
    """rp[128w+p] = h-dim held at (partition p, k-tile w) in hsT layout."""
    rp = np.empty(H, dtype=np.int64)
    for w in range(8):
        for p in range(P):
            rp[w * P + p] = 256 * (p // 32) + 32 * w + (p % 32)
    return rp


def build_nc(debug: bool = False):
    nc = bacc.Bacc("TRN2", target_bir_lowering=False, debug=False)

    xt = nc.declare_dram_parameter("xt", [IN, BL], BF16, isOutput=False)
    lab = nc.declare_dram_parameter("lab", [TB, 1], I32, isOutput=False)
    wft = nc.declare_dram_parameter("wft", [IN, E], BF16, isOutput=False)
    bf = nc.declare_dram_parameter("bf", [1, E], F32, isOutput=False)
    emb8 = nc.declare_dram_parameter("emb8", [V + 1, E], FP8, isOutput=False)
    wihe = nc.declare_dram_parameter("wihe", [E, G4], FP8, isOutput=False)
    wihf = nc.declare_dram_parameter("wihf", [E + P, G4], BF16, isOutput=False)
    onepad = nc.declare_dram_parameter("onepad", [BL, P], F32, isOutput=False)
    whh = nc.declare_dram_parameter("whh", [H, G4], BF16, isOutput=False)
    wout = nc.declare_dram_parameter("wout", [H, VPAD], BF16, isOutput=False)
    boutT = nc.declare_dram_parameter("boutT", [P, VS], F32, isOutput=False)
    logits8 = nc.declare_dram_parameter("logits8", [VS, NCH, P, 512], BF16,
                                        isOutput=True)
    if debug:
        d_feat = nc.declare_dram_parameter("d_feat", [BL, E], F32, isOutput=True)
        d_embT = nc.declare_dram_parameter("d_embT", [P, 4, TB], FP8, isOutput=True)
        d_fgr = nc.declare_dram_parameter("d_fgr", [P, 8, 512], F32, isOutput=True)
        d_ring = nc.declare_dram_parameter("d_ring", [NM, P, 8, 512], BF16,
                                           isOutput=True)
        d_hs = nc.declare_dram_parameter("d_hs", [T, P, 256], BF16, isOutput=True)

    with tile.TileContext(nc) as tc, contextlib.ExitStack() as top:
        pc = top.enter_context(tc.tile_pool(name="pc", bufs=1))
        ident = pc.tile([P, P], F32, tag="ident")
        make_identity(nc, ident[:])
        identb = pc.tile([P, P], BF16, tag="identb")
        make_identity(nc, identb[:])

        # PE warmup during the initial DMA wait: junk matmuls (transpose-mode
        # does not engage HAM) ramp the clock gate to 8/8 before real work
        with tc.tile_pool(name="pwu", bufs=2, space="PSUM") as pwu:
            for _ in range(55):
                wut = pwu.tile([P, P], F32, space="PSUM", tag="wut")
                nc.tensor.matmul(wut[:], identb[:], identb[:], start=True,
                                 stop=True)

        # persistent SBUF residents
        prec = top.enter_context(tc.tile_pool(name="prec", bufs=1))
        whh_sb = prec.tile([P, 8, G4], BF16, tag="whh")
        wihe_sb = prec.tile([P, 4, G4], FP8, tag="wihe")
        embT = prec.tile([P, 4, TB], FP8, tag="embT")
        hsT = prec.tile([P, T + 1, 256], BF16, tag="hsT")
        fgr = prec.tile([P, 8, 512], F32, tag="fgr")
        cst = prec.tile([P, 256], F32, tag="cst")
        bouT_sb = prec.tile([P, VS], F32, tag="bouT")

        # ---------------- phase 1+2 DMAs, critical-path first ----------------
        with (
            tc.tile_pool(name="pgath", bufs=1) as pgath,
            tc.tile_pool(name="pgat2", bufs=3) as pgat2,
            tc.tile_pool(name="ps1", bufs=2, space="PSUM") as ps1,
            tc.tile_pool(name="p2", bufs=1) as p2,
            tc.tile_pool(name="p2w", bufs=4) as p2w,
            tc.tile_pool(name="p2wf", bufs=8) as p2wf,
            tc.tile_pool(name="ps2", bufs=2, space="PSUM") as ps2,
            tc.tile_pool(name="ps2f", bufs=2, space="PSUM") as ps2f,
        ):
            lab_sb = pgath.tile([P, NM, 1], I32, tag="lab")
            nc.sync.dma_start(lab_sb[:], lab.rearrange("(m p) o -> p m o", p=P))
            xt_sb = p2.tile([P, IN // P, BL], BF16, tag="xt")
            nc.sync.dma_start(xt_sb[:], xt.rearrange("(k p) b -> p k b", p=P))
            bf_sb = p2.tile([BL, E], F32, tag="bf")
            nc.sync.dma_start(bf_sb[:], bf[0][None, :].broadcast_to((BL, E)))
            one_sb = p2.tile([BL, P], F32, tag="one")
            nc.sync.dma_start(one_sb[:], onepad[:])
            wft3 = wft.rearrange("(k p) e -> p k e", p=P)
            wft_sbs = []
            for kc in range(4):
                wft_sb = p2w.tile([P, 4, E], BF16, tag="wft")
                nc.sync.dma_start(wft_sb[:], wft3[:, ds(kc * 4, 4), :])
                wft_sbs.append(wft_sb)
            wihf3 = wihf.rearrange("(k p) c -> p k c", p=P)
            wihf_sbs = []
            for n in range(8):
                wihf_sb = p2wf.tile([P, 5, 512], BF16, tag="wihf")
                nc.sync.dma_start(wihf_sb[:], wihf3[:, :, ds(n * 512, 512)])
                wihf_sbs.append(wihf_sb)
            wihe3 = wihe.rearrange("(k p) c -> p k c", p=P)
            for k in range(4):
                nc.sync.dma_start(wihe_sb[:, k, :], wihe3[:, k, :])
            whh3 = whh.rearrange("(k p) c -> p k c", p=P)
            for k in range(8):
                nc.sync.dma_start(whh_sb[:, k, :], whh3[:, k, :])
            nc.sync.dma_start(bouT_sb[:], boutT[:])

            # gathers (indirect, SWDGE) + fp8->bf16 convert + PE transpose
            for m in range(NM):
                gath8 = pgat2.tile([P, E], FP8, tag="gath8")
                nc.gpsimd.indirect_dma_start(
                    out=gath8[:], out_offset=None, in_=emb8[:],
                    in_offset=bass.IndirectOffsetOnAxis(ap=lab_sb[:, m, :], axis=0),
                )
                gathb = pgat2.tile([P, E], BF16, tag="gathb")
                nc.scalar.copy(gathb[:], gath8[:])
                for k in range(4):
                    pst = ps1.tile([P, P], BF16, space="PSUM", tag="pst")
                    nc.tensor.transpose(pst[:], gathb[:, ds(k * P, P)], identb[:])
                    if k % 2 == 0:
                        nc.vector.tensor_copy(embT[:, k, ds(m * P, P)], pst[:])
                    else:
                        nc.scalar.copy(embT[:, k, ds(m * P, P)], pst[:])
            if debug:
                nc.sync.dma_start(d_embT[:], embT[:])

            # features
            psf = ps2f.tile([BL, E], F32, space="PSUM", tag="psf")
            nk = IN // P
            for kc in range(4):
                for k4 in range(4):
                    k = kc * 4 + k4
                    nc.tensor.matmul(psf[:], xt_sb[:, k, :], wft_sbs[kc][:, k4, :],
                                     start=(k == 0), stop=(k == nk - 1))
            fp = p2.tile([P, E + P], F32, tag="fp")
            nc.gpsimd.memset(fp[:], 0.0)
            nc.vector.tensor_add(fp[:BL, 0:E], psf[:], bf_sb[:])
            nc.vector.tensor_copy(fp[:BL, E:E + P], one_sb[:])
            if debug:
                nc.sync.dma_start(d_feat[:], fp[:BL, 0:E])

            # featT4: [E+pad chunk, kk, (tl, b)] -- batch cols replicated 4x
            featT4 = p2.tile([P, 5, P], BF16, tag="featT4")
            for kk in range(5):
                pst2 = ps2.tile([P, P], F32, space="PSUM", tag="pst2")
                nc.tensor.transpose(pst2[:], fp[:, ds(kk * P, P)], ident[:])
                for tl in range(4):
                    nc.vector.tensor_copy(featT4[:, kk, ds(32 * tl, 32)],
                                          pst2[:, 0:BL])

            # fgr GEMM: token-replicated feature gates (scaled by SGATE via wihf)
            for n in range(8):
                pfg = ps2f.tile([P, 512], F32, space="PSUM", tag="pfg")
                for kk in range(5):
                    nc.tensor.matmul(pfg[:], featT4[:, kk, :],
                                     wihf_sbs[n][:, kk, :],
                                     start=(kk == 0), stop=(kk == 4))
                nc.vector.tensor_copy(fgr[:, n, :], pfg[:])
            if debug:
                nc.sync.dma_start(d_fgr[:], fgr[:])

        # ---------------- gx GEMM (fp8 DoubleRow) + recurrence + ph5 ----------------
        wout3 = wout.rearrange("(k p) v -> p k v", p=P)
        with (
            tc.tile_pool(name="psG", bufs=2, space="PSUM") as psG,
            tc.tile_pool(name="pring", bufs=RB) as pring,
            tc.tile_pool(name="psA", bufs=2, space="PSUM") as psA,
            tc.tile_pool(name="psB", bufs=2, space="PSUM") as psB,
            tc.tile_pool(name="p4", bufs=2) as p4,
            tc.tile_pool(name="ps5", bufs=2, space="PSUM") as ps5p,
            tc.tile_pool(name="p5o", bufs=3) as p5o,
            tc.tile_pool(name="p5w", bufs=2) as p5w,
        ):
            rings = {}
            wchs = {}

            def gx_quarter(m, nq):
                """ring[m][:, 2*nq:2*nq+2, :] = DR-GEMM(embT chunk m, wihe) + fgr."""
                if nq == 0:
                    ring_new = pring.tile([P, 8, 512], BF16, tag="ring")
                    rings[m] = ring_new
                ring_t = rings[m]
                pges = []
                for n2 in range(2):
                    pge = psG.tile([P, 512], F32, space="PSUM", tag="pge")
                    pges.append(pge)
                for w2 in range(2):
                    for n2 in range(2):
                        n = nq * 2 + n2
                        nc.tensor.matmul(
                            pges[n2][:],
                            embT[:, ds(2 * w2, 2), ds(m * P, P)],
                            wihe_sb[:, ds(2 * w2, 2), ds(n * 512, 512)],
                            start=(w2 == 0), stop=(w2 == 1), perf_mode=DR,
                            skip_group_check=True)
                for n2 in range(2):
                    n = nq * 2 + n2
                    nc.vector.tensor_add(ring_t[:, n, :], pges[n2][:],
                                         fgr[:, n, :])
                if debug and nq == 3:
                    nc.sync.dma_start(d_ring[m], ring_t[:])

            def gx_seed(t, pa, pb, stop=False):
                ring_t = rings[t // 4]
                tl = t % 4
                for a, ps_ in ((0, pa), (1, pb)):
                    for q in range(4):
                        nc.tensor.matmul(
                            ps_[ds(32 * q, 32), :], identb[:, ds(32 * tl, 32)],
                            ring_t[:, 4 * a + q, :], start=True, stop=stop,
                            tile_position=(0, 32 * q), skip_group_check=True)

            def rec_bank(t, a, ps_):
                for w in range(8):
                    for q in range(4):
                        nc.tensor.matmul(
                            ps_[ds(32 * q, 32), :],
                            hsT[:, t, ds(32 * w, 32)],
                            whh_sb[:, w, ds((4 * a + q) * 512, 512)],
                            start=False, stop=(w == 7), tile_position=(0, 32 * q),
                            skip_group_check=True)

            def ph5_unit(vs, ch, pools=None):
                """One [128 vocab x N tb] output tile: 8 mms + bias act + DMA."""
                psum_p, out_p = pools or (ps5p, p5o)
                vc, vl = divmod(vs, 4)
                N = 512 if ch < 2 else 256
                pu = psum_p.tile([P, 512], F32, space="PSUM", tag="pu")
                for w in range(8):
                    nc.tensor.matmul(
                        pu[:, 0:N],
                        wchs[vc][:, w, ds(128 * vl, P)],
                        hsT[:, ds(1 + 16 * ch, N // 32), ds(32 * w, 32)],
                        start=(w == 0), stop=(w == 7))
                osb = out_p.tile([P, 512], BF16, tag="osb")
                nc.scalar.activation(osb[:, 0:N], pu[:, 0:N], Ident,
                                     bias=bouT_sb[:, ds(vs, 1)])
                nc.sync.dma_start(logits8[vs, ch, :, 0:N], osb[:, 0:N])

            def wch_dma(vc, pool=None):
                wch = (pool or p5w).tile([P, 8, 512], BF16, tag="wch")
                wchs[vc] = wch
                nc.sync.dma_start(wch[:], wout3[:, :, ds(512 * vc, 512)])

            nc.gpsimd.memset(cst[:], 0.0)

            # preload ring chunk 0; chunks 1..9 stream as quarters in-loop
            for nq in range(4):
                gx_quarter(0, nq)
            wch_dma(0)

            pa = psA.tile([P, 512], F32, space="PSUM", tag="pa")
            pb = psB.tile([P, 512], F32, space="PSUM", tag="pb")
            gx_seed(0, pa, pb, stop=True)

            # sliver layout for the bank-B tail
            SLV = ((0, 64), (64, 192))
            for t in range(T):
                # ph5 ch0 units from t=16 (hsT slots 1..16 ready), one per step
                if t >= 16 and (t - 16) < 24:
                    vs = t - 16
                    if vs % 4 == 0 and vs // 4 + 1 < 6:
                        wch_dma(vs // 4 + 1)
                    ph5_unit(vs, 0)
                if t > 0:
                    rec_bank(t, 0, pa)
                # bank A activations: sif = sig([i|f]); csf = sf * c
                sif = p4.tile([P, 512], BF16, tag="sif")
                csf = p4.tile([P, 256], F32, tag="csf")
                nc.scalar.activation(sif[:], pa[:], Sig, scale=ISG)
                nc.gpsimd.tensor_mul(csf[:], sif[:, 256:512], cst[:])

                if t > 0:
                    rec_bank(t, 1, pb)
                # seeds for next step fill the PE while the B-side tail runs
                if t + 1 < T:
                    pa2 = psA.tile([P, 512], F32, space="PSUM", tag="pa")
                    pb2 = psB.tile([P, 512], F32, space="PSUM", tag="pb")
                    gx_seed(t + 1, pa2, pb2)
                # bank B tail in slivers: tg = tanh(g); t3 = si*tg; c = csf+t3;
                # tcc = tanh(c); h = so*tcc; hsT[t+1] sliver = blockT(h sliver)
                tg = p4.tile([P, 256], BF16, tag="tg")
                so = p4.tile([P, 256], BF16, tag="so")
                tcc = p4.tile([P, 256], BF16, tag="tcc")
                t3 = p4.tile([P, 256], F32, tag="t3")
                h128 = p4.tile([P, 256], BF16, tag="h128")
                for (o, w_) in SLV:
                    nc.scalar.activation(tg[:, ds(o, w_)], pb[:, ds(o, w_)],
                                         Tanh, scale=ISG)
                nc.scalar.activation(so[:], pb[:, 256:512], Sig, scale=ISG)
                for (o, w_) in SLV:
                    sl = ds(o, w_)
                    nc.vector.tensor_mul(t3[:, sl], sif[:, sl], tg[:, sl])
                    nc.vector.tensor_add(cst[:, sl], csf[:, sl], t3[:, sl])
                    nc.scalar.activation(tcc[:, sl], cst[:, sl], Tanh)
                    nc.vector.tensor_mul(h128[:, sl], so[:, sl], tcc[:, sl])
                    nc.vector.transpose(hsT[:, t + 1, sl], h128[:, sl])
                if debug:
                    nc.sync.dma_start(d_hs[t], h128[:])
                # stream gx quarters, 2/step: chunk 1+t//2 over t=0..15, c9 at 16-17
                if t < 16:
                    m2 = 1 + t // 2
                    gx_quarter(m2, 2 * (t % 2))
                    gx_quarter(m2, 2 * (t % 2) + 1)
                elif t < 18:
                    gx_quarter(9, 2 * (t - 16))
                    gx_quarter(9, 2 * (t - 16) + 1)
                if t + 1 < T:
                    pa, pb = pa2, pb2

        # ---------------- ph5 drain (remaining units, deep pipelining) ----------------
        with (
            tc.tile_pool(name="ps5d", bufs=4, space="PSUM") as ps5d,
            tc.tile_pool(name="p5od", bufs=8) as p5od,
            tc.tile_pool(name="p5wd", bufs=3) as p5wd,
        ):
            for vc in range(VPAD // 512):
                wch_dma(vc, pool=p5wd)
                for vl in range(4):
                    vs = 4 * vc + vl
                    for ch in range(NCH):
                        if ch == 0 and vs < VS - 16:
                            continue  # done in-loop
                        ph5_unit(vs, ch, pools=(ps5d, p5od))

    nc.finalize()
    return nc


_NC_CACHE: dict = {}


def _get_nc(debug: bool = False):
    key = bool(debug)
    if key not in _NC_CACHE:
        _NC_CACHE[key] = build_nc(debug=key)
    return _NC_CACHE[key]


def host_prep(inputs: dict) -> list[dict]:
    """Shard + lay out inputs for the 8 cores."""
    X = np.asarray(inputs["X"], dtype=np.float32)
    labels = np.asarray(inputs["labels"])
    W_f = np.asarray(inputs["W_f"], dtype=np.float32)
    b_f = np.asarray(inputs["b_f"], dtype=np.float32)
    emb = np.asarray(inputs["emb"], dtype=np.float32)
    W_ih = np.asarray(inputs["W_ih"], dtype=np.float32)
    W_hh = np.asarray(inputs["W_hh"], dtype=np.float32)
    b_ih = np.asarray(inputs["b_ih"], dtype=np.float32)
    b_hh = np.asarray(inputs["b_hh"], dtype=np.float32)
    W_out = np.asarray(inputs["W_out"], dtype=np.float32)
    b_out = np.asarray(inputs["b_out"], dtype=np.float32)

    perm = gate_perm()
    rp = row_perm()
    bff = ml_dtypes.bfloat16
    f8 = ml_dtypes.float8_e4m3fn
    wft = np.ascontiguousarray(W_f.T).astype(bff)                      # [IN, E]
    emb8 = np.clip(emb * SE, -240.0, 240.0).astype(f8)                 # [V+1, E]
    wihe = np.ascontiguousarray(W_ih[:, E:].T[:, perm] * SE).astype(f8)
    wihf_aug = np.zeros((E + P, G4), dtype=np.float32)
    wihf_aug[:E] = W_ih[:, :E].T[:, perm] * SGATE
    wihf_aug[E] = (b_ih + b_hh)[perm] * SGATE
    wihf_aug = wihf_aug.astype(bff)
    whh = np.ascontiguousarray((W_hh.T * SGATE)[rp][:, perm]).astype(bff)
    wout_p = np.zeros((H, VPAD), dtype=np.float32)
    wout_p[:, :V] = W_out.T[rp]
    wout_p = wout_p.astype(bff)
    boutT = np.zeros((P, VS), dtype=np.float32)
    boutT.T.reshape(-1)[:V] = b_out
    onepad = np.zeros((BL, P), dtype=np.float32)
    onepad[:, 0] = 1.0

    shared = {
        "wft": wft, "bf": b_f[None, :], "emb8": emb8, "wihe": wihe,
        "wihf": wihf_aug, "onepad": onepad, "whh": whh, "wout": wout_p,
        "boutT": boutT,
    }
    shifted = np.roll(labels, 1, axis=1)                               # [B, T]
    in_maps = []
    for c in range(NCORES):
        s = slice(c * BL, (c + 1) * BL)
        xt = np.ascontiguousarray(X[s].T).astype(bff)                  # [IN, 32]
        lab = np.ascontiguousarray(shifted[s].T.reshape(TB, 1)).astype(np.int32)
        in_maps.append({**shared, "xt": xt, "lab": lab})
    return in_maps


def unpack_logits(raw: np.ndarray) -> np.ndarray:
    """[VS, NCH, 128, 512] bf16 -> [BL, T, V] fp32."""
    arr = np.asarray(raw).astype(np.float32)         # [40, 3, 128, 512]
    flat = arr.transpose(1, 3, 0, 2).reshape(NCH * 512, VPAD)  # [tb', v]
    flat = flat[:TB, :V]                             # [1280, 5000]
    return np.ascontiguousarray(
        flat.reshape(T, BL, V).transpose(1, 0, 2))   # [32, 40, 5000]


def run(inputs: dict, debug: bool = False, trace: bool = False):
    nc = _get_nc(debug=debug)
    in_maps = host_prep(inputs)
    r = run_bass_kernel_spmd(nc, in_maps, core_ids=list(range(NCORES)), trace=trace)
    outs = [unpack_logits(r.results[c]["logits8"]) for c in range(NCORES)]
    out = np.concatenate(outs, axis=0)
    return out, r


def kernel(**inputs) -> np.ndarray:
    out, _ = run(inputs, debug=False, trace=False)
    return out


if __name__ == "__main__":
    rng = np.random.default_rng(0)
    fake = {
        "X": rng.standard_normal((B, IN)).astype(np.float32),
        "labels": rng.integers(0, V, size=(B, T)).astype(np.int64),
        "W_f": (rng.standard_normal((E, IN)) * 0.02).astype(np.float32),
        "b_f": np.zeros(E, np.float32),
        "emb": (rng.standard_normal((V + 1, E)) * 0.02).astype(np.float32),
        "W_ih": (rng.standard_normal((G4, 2 * E)) * 0.02).astype(np.float32),
        "W_hh": (rng.standard_normal((G4, H)) * 0.02).astype(np.float32),
        "b_ih": np.zeros(G4, np.float32),
        "b_hh": np.zeros(G4, np.float32),
        "W_out": (rng.standard_normal((V, H)) * 0.02).astype(np.float32),
        "b_out": np.zeros(V, np.float32),
    }
    out = kernel(**fake)
    print("out", out.shape, out.dtype, float(np.abs(out).max()))


# revision 38
# speedup vs baseline: 1.1865x; 1.0149x over previous
"""CCRNN Trainium2 kernel: feature MLP + embedding lookup + 40-step LSTM + vocab projection.

Sharding: data-parallel over batch B=256 -> 8 cores x 32. Weights replicated.

v3 design (fp8-DoubleRow embedding-gate GEMM + select-seeded recurrence):
  ph1: gather shifted embeddings as fp8 rows (host pre-quantized emb8 = fp8(emb*64)),
       bf16-bounce PE-transpose -> embT fp8 [128, 4, 1280].
  ph2: features = X @ W_f.T + b_f -> featT; featT4 = featT col-replicated x4
       (token layout (t%4, b)); fgr [128, 8, 512] f32 = 4096*(feat@W_ih_feat.T
       + b_ih + b_hh) in token-replicated layout via M=128 GEMM.
  gx GEMM (chunks m=0..9 of 128 tokens; c0 pre-loop, rest streamed as quarter
       chunks 2/step into the loop's tail windows):
       pge [128 tok, 512] = embT-pairs x wihe-pairs (fp8 DoubleRow, 2 mms/chunk-n);
       ring[m][:, n, :] bf16 = pge + fgr[:, n, :]  (DVE add; ring pool bufs=6).
  loop t=0..39 (PSUM banks A/B [128(32q+b), 512]):
    seed: 4 col-tiled select-mms/bank (identb[:, 32*(t%4):+32] stationary) copy
    ring rows into psum (start=True); rec-mm (hsT x whh bf16, t>=1) accumulates;
    activations read PSUM with scale 2^-12; bank-B tail in (64, 192) slivers so
    hsT cols 0:64 land early and rec(t+1) w0..1 starts during the rest of the
    tail; one ph5 ch0 unit per step from t=16 fills the tail's PE window.
  ph5: logits bf16 [40vt, 3ch, 128, 512] = (psum * 2^-12 + b_out) (wout bf16
       streamed); 24 ch0 units run in-loop, the rest drain after with deep
       psum/output pipelining; host reorder.

Gate column permutation (newcol -> orig), n = a*4+q in 0..7, j in 0..511:
  a=0: [i_q | f_q], a=1: [g_q | o_q]  (torch gate order i,f,g,o; quarter q = H rows
  256q..256q+255).
Row (K-dim) permutation for hsT-layout: rp[128w+p] = 256*(p//32) + 32*w + (p%32).
"""

import os
import sys
import types
import contextlib

import numpy as np
import ml_dtypes

# ---- environment bootstrap (works in a bare dir; paths are machine-level) ----
for _p in ("/root/.axon_site", "/root/.axon_site/_ro/trn_rl_repo",
           "/root/.axon_site/_ro/pypackages"):
    if _p not in sys.path and os.path.isdir(_p):
        sys.path.insert(0, _p)

# shim the NTFF profile hook if the image's antenv lacks it (needed for trace=True)
try:
    from antenv.axon_hooks import get_axon_ntff_profile_hook  # noqa: F401
except ImportError:
    try:
        from trn_agent_boot.trn_boot import _ntff_profile_via_ctypes
        _m = types.ModuleType("antenv.axon_hooks")
        _hook = _ntff_profile_via_ctypes("/opt/axon/libaxon_pjrt.so")
        _m.get_axon_ntff_profile_hook = lambda: _hook
        _m.set_axon_ntff_profile_hook = lambda h: None
        import antenv  # noqa: F401
        sys.modules["antenv.axon_hooks"] = _m
    except Exception:
        pass

import concourse.bass as bass
import concourse.tile as tile
from concourse import mybir, bacc
from concourse import bass_utils
bass_utils.upload_artifacts = lambda tmpdir: f"file://{tmpdir}"
from concourse.bass import ds
from concourse.bass_utils import run_bass_kernel_spmd
from concourse.masks import make_identity

P = 128
B, T = 256, 40
IN, E, H, V = 2048, 512, 1024, 5000
G4 = 4 * H
NCORES = 8
BL = B // NCORES            # 32 per core
TB = T * BL                 # 1280 (t*32+b) rows
NM = TB // P                # 10 token chunks of 128
F32 = mybir.dt.float32
BF16 = mybir.dt.bfloat16
FP8 = mybir.dt.float8e4
I32 = mybir.dt.int32
DR = mybir.MatmulPerfMode.DoubleRow
Sig = mybir.ActivationFunctionType.Sigmoid
Tanh = mybir.ActivationFunctionType.Tanh
Copy = mybir.ActivationFunctionType.Copy
Ident = mybir.ActivationFunctionType.Identity

SGATE = 4096.0              # psum gate scale
ISG = 1.0 / SGATE
SE = 64.0                   # fp8 emb / wihe scale
VS = 40                     # vocab tiles of 128 (pad 5000 -> 5120)
VPAD = VS * P               # 5120
NCH = 3                     # (t,b) chunks: 512, 512, 256
RB = 6                      # gx ring buffers (chunks of 4 timesteps)


def gate_perm() -> np.ndarray:
    """newcol -> orig index over the 4096 gate dim."""
    perm = np.empty(G4, dtype=np.int64)
    for n in range(8):
        a, q = divmod(n, 4)
        for j in range(512):
            g = (0 if j < 256 else 1) + 2 * a          # i/f for bank0, g/o for bank1
            jj = j if j < 256 else j - 256
            perm[n * 512 + j] = g * H + q * 256 + jj
    return perm


def row_perm() -> np.ndarray:
    """rp[128w+p] = h-dim held at (partition p, k-tile w) in hsT layout."""
    rp = np.empty(H, dtype=np.int64)
    for w in range(8):
        for p in range(P):
            rp[w * P + p] = 256 * (p // 32) + 32 * w + (p % 32)
    return rp


def build_nc(debug: bool = False):
    nc = bacc.Bacc("TRN2", target_bir_lowering=False, debug=False)

    xt = nc.declare_dram_parameter("xt", [IN, BL], BF16, isOutput=False)
    lab = nc.declare_dram_parameter("lab", [TB, 1], I32, isOutput=False)
    wft = nc.declare_dram_parameter("wft", [IN, E], BF16, isOutput=False)
    bf = nc.declare_dram_parameter("bf", [1, E], F32, isOutput=False)
    emb8 = nc.declare_dram_parameter("emb8", [V + 1, E], FP8, isOutput=False)
    wihe = nc.declare_dram_parameter("wihe", [E, G4], FP8, isOutput=False)
    wihf = nc.declare_dram_parameter("wihf", [E + P, G4], BF16, isOutput=False)
    onepad = nc.declare_dram_parameter("onepad", [BL, P], F32, isOutput=False)
    whh = nc.declare_dram_parameter("whh", [H, G4], BF16, isOutput=False)
    wout = nc.declare_dram_parameter("wout", [H, VPAD], BF16, isOutput=False)
    boutT = nc.declare_dram_parameter("boutT", [P, VS], F32, isOutput=False)
    logits8 = nc.declare_dram_parameter("logits8", [VS, NCH, P, 512], BF16,
                                        isOutput=True)
    if debug:
        d_feat = nc.declare_dram_parameter("d_feat", [BL, E], F32, isOutput=True)
        d_embT = nc.declare_dram_parameter("d_embT", [P, 4, TB], FP8, isOutput=True)
        d_fgr = nc.declare_dram_parameter("d_fgr", [P, 8, 512], F32, isOutput=True)
        d_ring = nc.declare_dram_parameter("d_ring", [NM, P, 8, 512], BF16,
                                           isOutput=True)
        d_hs = nc.declare_dram_parameter("d_hs", [T, P, 256], BF16, isOutput=True)

    with tile.TileContext(nc) as tc, contextlib.ExitStack() as top:
        pc = top.enter_context(tc.tile_pool(name="pc", bufs=1))
        ident = pc.tile([P, P], F32, tag="ident")
        make_identity(nc, ident[:])
        identb = pc.tile([P, P], BF16, tag="identb")
        make_identity(nc, identb[:])

        # PE warmup during the initial DMA wait: junk matmuls (transpose-mode
        # does not engage HAM) ramp the clock gate to 8/8 before real work
        with tc.tile_pool(name="pwu", bufs=2, space="PSUM") as pwu:
            for _ in range(55):
                wut = pwu.tile([P, P], F32, space="PSUM", tag="wut")
                nc.tensor.matmul(wut[:], identb[:], identb[:], start=True,
                                 stop=True)

        # persistent SBUF residents
        prec = top.enter_context(tc.tile_pool(name="prec", bufs=1))
        whh_sb = prec.tile([P, 8, G4], BF16, tag="whh")
        wihe_sb = prec.tile([P, 4, G4], FP8, tag="wihe")
        embT = prec.tile([P, 4, TB], FP8, tag="embT")
        hsT = prec.tile([P, T + 1, 256], BF16, tag="hsT")
        fgr = prec.tile([P, 8, 512], F32, tag="fgr")
        cst = prec.tile([P, 256], F32, tag="cst")
        bouT_sb = prec.tile([P, VS], F32, tag="bouT")

        # ---------------- phase 1+2 DMAs, critical-path first ----------------
        with (
            tc.tile_pool(name="pgath", bufs=1) as pgath,
            tc.tile_pool(name="pgat2", bufs=3) as pgat2,
            tc.tile_pool(name="ps1", bufs=2, space="PSUM") as ps1,
            tc.tile_pool(name="p2", bufs=1) as p2,
            tc.tile_pool(name="p2w", bufs=4) as p2w,
            tc.tile_pool(name="p2wf", bufs=8) as p2wf,
            tc.tile_pool(name="ps2", bufs=2, space="PSUM") as ps2,
            tc.tile_pool(name="ps2f", bufs=2, space="PSUM") as ps2f,
        ):
            lab_sb = pgath.tile([P, NM, 1], I32, tag="lab")
            nc.sync.dma_start(lab_sb[:], lab.rearrange("(m p) o -> p m o", p=P))
            xt_sb = p2.tile([P, IN // P, BL], BF16, tag="xt")
            nc.sync.dma_start(xt_sb[:], xt.rearrange("(k p) b -> p k b", p=P))
            bf_sb = p2.tile([BL, E], F32, tag="bf")
            nc.sync.dma_start(bf_sb[:], bf[0][None, :].broadcast_to((BL, E)))
            one_sb = p2.tile([BL, P], F32, tag="one")
            nc.sync.dma_start(one_sb[:], onepad[:])
            wft3 = wft.rearrange("(k p) e -> p k e", p=P)
            wft_sbs = []
            for kc in range(4):
                wft_sb = p2w.tile([P, 4, E], BF16, tag="wft")
                nc.sync.dma_start(wft_sb[:], wft3[:, ds(kc * 4, 4), :])
                wft_sbs.append(wft_sb)
            wihf3 = wihf.rearrange("(k p) c -> p k c", p=P)
            wihf_sbs = []
            for n in range(8):
                wihf_sb = p2wf.tile([P, 5, 512], BF16, tag="wihf")
                nc.sync.dma_start(wihf_sb[:], wihf3[:, :, ds(n * 512, 512)])
                wihf_sbs.append(wihf_sb)
            wihe3 = wihe.rearrange("(k p) c -> p k c", p=P)
            for k in range(4):
                nc.sync.dma_start(wihe_sb[:, k, :], wihe3[:, k, :])
            whh3 = whh.rearrange("(k p) c -> p k c", p=P)
            for k in range(8):
                nc.sync.dma_start(whh_sb[:, k, :], whh3[:, k, :])
            nc.sync.dma_start(bouT_sb[:], boutT[:])

            # issue all gathers up-front (SWDGE, one persistent landing tile)
            fp = p2.tile([P, E + P], F32, tag="fp")
            nc.gpsimd.memset(fp[:], 0.0)
            gath8a = pgath.tile([P, NM, E], FP8, tag="gath8a")
            for m in range(NM):
                nc.gpsimd.indirect_dma_start(
                    out=gath8a[:, m, :], out_offset=None, in_=emb8[:],
                    in_offset=bass.IndirectOffsetOnAxis(ap=lab_sb[:, m, :], axis=0),
                )

            # features (emitted before embT work so each engine FIFO serves
            # the fgs-critical path first)
            psf = ps2f.tile([BL, E], F32, space="PSUM", tag="psf")
            nk = IN // P
            for kc in range(4):
                for k4 in range(4):
                    k = kc * 4 + k4
                    nc.tensor.matmul(psf[:], xt_sb[:, k, :], wft_sbs[kc][:, k4, :],
                                     start=(k == 0), stop=(k == nk - 1))
            nc.vector.tensor_add(fp[:BL, 0:E], psf[:], bf_sb[:])
            nc.vector.tensor_copy(fp[:BL, E:E + P], one_sb[:])
            if debug:
                nc.sync.dma_start(d_feat[:], fp[:BL, 0:E])

            # featT4: [E+pad chunk, kk, (tl, b)] -- batch cols replicated 4x
            featT4 = p2.tile([P, 5, P], BF16, tag="featT4")
            for kk in range(5):
                pst2 = ps2.tile([P, P], F32, space="PSUM", tag="pst2")
                nc.tensor.transpose(pst2[:], fp[:, ds(kk * P, P)], ident[:])
                for tl in range(4):
                    nc.vector.tensor_copy(featT4[:, kk, ds(32 * tl, 32)],
                                          pst2[:, 0:BL])

            # fgr GEMM: token-replicated feature gates (scaled by SGATE via wihf)
            for n in range(8):
                pfg = ps2f.tile([P, 512], F32, space="PSUM", tag="pfg")
                for kk in range(5):
                    nc.tensor.matmul(pfg[:], featT4[:, kk, :],
                                     wihf_sbs[n][:, kk, :],
                                     start=(kk == 0), stop=(kk == 4))
                nc.vector.tensor_copy(fgr[:, n, :], pfg[:])
            if debug:
                nc.sync.dma_start(d_fgr[:], fgr[:])

            # embT prep: fp8->bf16 convert + PE transpose + fp8 store
            for m in range(NM):
                gathb = pgat2.tile([P, E], BF16, tag="gathb")
                nc.scalar.copy(gathb[:], gath8a[:, m, :])
                for k in range(4):
                    pst = ps1.tile([P, P], BF16, space="PSUM", tag="pst")
                    nc.tensor.transpose(pst[:], gathb[:, ds(k * P, P)], identb[:])
                    if k % 2 == 0:
                        nc.vector.tensor_copy(embT[:, k, ds(m * P, P)], pst[:])
                    else:
                        nc.scalar.copy(embT[:, k, ds(m * P, P)], pst[:])
            if debug:
                nc.sync.dma_start(d_embT[:], embT[:])

        # ---------------- gx GEMM (fp8 DoubleRow) + recurrence + ph5 ----------------
        wout3 = wout.rearrange("(k p) v -> p k v", p=P)
        with (
            tc.tile_pool(name="psG", bufs=2, space="PSUM") as psG,
            tc.tile_pool(name="pring", bufs=RB) as pring,
            tc.tile_pool(name="psA", bufs=2, space="PSUM") as psA,
            tc.tile_pool(name="psB", bufs=2, space="PSUM") as psB,
            tc.tile_pool(name="p4", bufs=2) as p4,
            tc.tile_pool(name="ps5", bufs=2, space="PSUM") as ps5p,
            tc.tile_pool(name="p5o", bufs=3) as p5o,
            tc.tile_pool(name="p5w", bufs=2) as p5w,
        ):
            rings = {}
            wchs = {}

            def gx_quarter(m, nq):
                """ring[m][:, 2*nq:2*nq+2, :] = DR-GEMM(embT chunk m, wihe) + fgr."""
                if nq == 0:
                    ring_new = pring.tile([P, 8, 512], BF16, tag="ring")
                    rings[m] = ring_new
                ring_t = rings[m]
                pges = []
                for n2 in range(2):
                    pge = psG.tile([P, 512], F32, space="PSUM", tag="pge")
                    pges.append(pge)
                for w2 in range(2):
                    for n2 in range(2):
                        n = nq * 2 + n2
                        nc.tensor.matmul(
                            pges[n2][:],
                            embT[:, ds(2 * w2, 2), ds(m * P, P)],
                            wihe_sb[:, ds(2 * w2, 2), ds(n * 512, 512)],
                            start=(w2 == 0), stop=(w2 == 1), perf_mode=DR,
                            skip_group_check=True)
                for n2 in range(2):
                    n = nq * 2 + n2
                    nc.vector.tensor_add(ring_t[:, n, :], pges[n2][:],
                                         fgr[:, n, :])
                if debug and nq == 3:
                    nc.sync.dma_start(d_ring[m], ring_t[:])

            def gx_seed(t, pa, pb, stop=False):
                ring_t = rings[t // 4]
                tl = t % 4
                for a, ps_ in ((0, pa), (1, pb)):
                    for q in range(4):
                        nc.tensor.matmul(
                            ps_[ds(32 * q, 32), :], identb[:, ds(32 * tl, 32)],
                            ring_t[:, 4 * a + q, :], start=True, stop=stop,
                            tile_position=(0, 32 * q), skip_group_check=True)

            def rec_bank(t, a, ps_):
                for w in range(8):
                    for q in range(4):
                        nc.tensor.matmul(
                            ps_[ds(32 * q, 32), :],
                            hsT[:, t, ds(32 * w, 32)],
                            whh_sb[:, w, ds((4 * a + q) * 512, 512)],
                            start=False, stop=(w == 7), tile_position=(0, 32 * q),
                            skip_group_check=True)

            def ph5_unit(vs, ch, pools=None):
                """One [128 vocab x N tb] output tile: 8 mms + bias act + DMA."""
                psum_p, out_p = pools or (ps5p, p5o)
                vc, vl = divmod(vs, 4)
                N = 512 if ch < 2 else 256
                pu = psum_p.tile([P, 512], F32, space="PSUM", tag="pu")
                for w in range(8):
                    nc.tensor.matmul(
                        pu[:, 0:N],
                        wchs[vc][:, w, ds(128 * vl, P)],
                        hsT[:, ds(1 + 16 * ch, N // 32), ds(32 * w, 32)],
                        start=(w == 0), stop=(w == 7))
                osb = out_p.tile([P, 512], BF16, tag="osb")
                nc.scalar.activation(osb[:, 0:N], pu[:, 0:N], Ident,
                                     bias=bouT_sb[:, ds(vs, 1)])
                nc.sync.dma_start(logits8[vs, ch, :, 0:N], osb[:, 0:N])

            def wch_dma(vc, pool=None):
                wch = (pool or p5w).tile([P, 8, 512], BF16, tag="wch")
                wchs[vc] = wch
                nc.sync.dma_start(wch[:], wout3[:, :, ds(512 * vc, 512)])

            nc.gpsimd.memset(cst[:], 0.0)

            # preload ring chunk 0; chunks 1..9 stream as quarters in-loop
            for nq in range(4):
                gx_quarter(0, nq)
            wch_dma(0)

            pa = psA.tile([P, 512], F32, space="PSUM", tag="pa")
            pb = psB.tile([P, 512], F32, space="PSUM", tag="pb")
            gx_seed(0, pa, pb, stop=True)

            # sliver layout for the bank-B tail
            SLV = ((0, 64), (64, 192))
            for t in range(T):
                # ph5 ch0 units from t=16 (hsT slots 1..16 ready), one per step
                if t >= 16 and (t - 16) < 24:
                    vs = t - 16
                    if vs % 4 == 0 and vs // 4 + 1 < 6:
                        wch_dma(vs // 4 + 1)
                    ph5_unit(vs, 0)
                if t > 0:
                    rec_bank(t, 0, pa)
                # bank A activations: sif = sig([i|f]); csf = sf * c
                sif = p4.tile([P, 512], BF16, tag="sif")
                csf = p4.tile([P, 256], F32, tag="csf")
                nc.scalar.activation(sif[:], pa[:], Sig, scale=ISG)
                nc.gpsimd.tensor_mul(csf[:], sif[:, 256:512], cst[:])

                if t > 0:
                    rec_bank(t, 1, pb)
                # seeds for next step fill the PE while the B-side tail runs
                if t + 1 < T:
                    pa2 = psA.tile([P, 512], F32, space="PSUM", tag="pa")
                    pb2 = psB.tile([P, 512], F32, space="PSUM", tag="pb")
                    gx_seed(t + 1, pa2, pb2)
                # bank B tail in slivers: tg = tanh(g); t3 = si*tg; c = csf+t3;
                # tcc = tanh(c); h = so*tcc; hsT[t+1] sliver = blockT(h sliver)
                tg = p4.tile([P, 256], BF16, tag="tg")
                so = p4.tile([P, 256], BF16, tag="so")
                tcc = p4.tile([P, 256], BF16, tag="tcc")
                t3 = p4.tile([P, 256], F32, tag="t3")
                h128 = p4.tile([P, 256], BF16, tag="h128")
                for (o, w_) in SLV:
                    nc.scalar.activation(tg[:, ds(o, w_)], pb[:, ds(o, w_)],
                                         Tanh, scale=ISG)
                nc.scalar.activation(so[:], pb[:, 256:512], Sig, scale=ISG)
                for (o, w_) in SLV:
                    sl = ds(o, w_)
                    nc.vector.tensor_mul(t3[:, sl], sif[:, sl], tg[:, sl])
                    nc.vector.tensor_add(cst[:, sl], csf[:, sl], t3[:, sl])
                    nc.scalar.activation(tcc[:, sl], cst[:, sl], Tanh)
                    nc.vector.tensor_mul(h128[:, sl], so[:, sl], tcc[:, sl])
                    nc.vector.transpose(hsT[:, t + 1, sl], h128[:, sl])
                if debug:
                    nc.sync.dma_start(d_hs[t], h128[:])
                # stream gx quarters, 2/step: chunk 1+t//2 over t=0..15, c9 at 16-17
                if t < 16:
                    m2 = 1 + t // 2
                    gx_quarter(m2, 2 * (t % 2))
                    gx_quarter(m2, 2 * (t % 2) + 1)
                elif t < 18:
                    gx_quarter(9, 2 * (t - 16))
                    gx_quarter(9, 2 * (t - 16) + 1)
                if t + 1 < T:
                    pa, pb = pa2, pb2

        # ---------------- ph5 drain (remaining units, deep pipelining) ----------------
        with (
            tc.tile_pool(name="ps5d", bufs=4, space="PSUM") as ps5d,
            tc.tile_pool(name="p5od", bufs=8) as p5od,
            tc.tile_pool(name="p5wd", bufs=3) as p5wd,
        ):
            for vc in range(VPAD // 512):
                wch_dma(vc, pool=p5wd)
                for vl in range(4):
                    vs = 4 * vc + vl
                    for ch in range(NCH):
                        if ch == 0 and vs < VS - 16:
                            continue  # done in-loop
                        ph5_unit(vs, ch, pools=(ps5d, p5od))

    nc.finalize()
    return nc


_NC_CACHE: dict = {}


def _get_nc(debug: bool = False):
    key = bool(debug)
    if key not in _NC_CACHE:
        _NC_CACHE[key] = build_nc(debug=key)
    return _NC_CACHE[key]


def host_prep(inputs: dict) -> list[dict]:
    """Shard + lay out inputs for the 8 cores."""
    X = np.asarray(inputs["X"], dtype=np.float32)
    labels = np.asarray(inputs["labels"])
    W_f = np.asarray(inputs["W_f"], dtype=np.float32)
    b_f = np.asarray(inputs["b_f"], dtype=np.float32)
    emb = np.asarray(inputs["emb"], dtype=np.float32)
    W_ih = np.asarray(inputs["W_ih"], dtype=np.float32)
    W_hh = np.asarray(inputs["W_hh"], dtype=np.float32)
    b_ih = np.asarray(inputs["b_ih"], dtype=np.float32)
    b_hh = np.asarray(inputs["b_hh"], dtype=np.float32)
    W_out = np.asarray(inputs["W_out"], dtype=np.float32)
    b_out = np.asarray(inputs["b_out"], dtype=np.float32)

    perm = gate_perm()
    rp = row_perm()
    bff = ml_dtypes.bfloat16
    f8 = ml_dtypes.float8_e4m3fn
    wft = np.ascontiguousarray(W_f.T).astype(bff)                      # [IN, E]
    emb8 = np.clip(emb * SE, -240.0, 240.0).astype(f8)                 # [V+1, E]
    wihe = np.ascontiguousarray(W_ih[:, E:].T[:, perm] * SE).astype(f8)
    wihf_aug = np.zeros((E + P, G4), dtype=np.float32)
    wihf_aug[:E] = W_ih[:, :E].T[:, perm] * SGATE
    wihf_aug[E] = (b_ih + b_hh)[perm] * SGATE
    wihf_aug = wihf_aug.astype(bff)
    whh = np.ascontiguousarray((W_hh.T * SGATE)[rp][:, perm]).astype(bff)
    wout_p = np.zeros((H, VPAD), dtype=np.float32)
    wout_p[:, :V] = W_out.T[rp]
    wout_p = wout_p.astype(bff)
    boutT = np.zeros((P, VS), dtype=np.float32)
    boutT.T.reshape(-1)[:V] = b_out
    onepad = np.zeros((BL, P), dtype=np.float32)
    onepad[:, 0] = 1.0

    shared = {
        "wft": wft, "bf": b_f[None, :], "emb8": emb8, "wihe": wihe,
        "wihf": wihf_aug, "onepad": onepad, "whh": whh, "wout": wout_p,
        "boutT": boutT,
    }
    shifted = np.roll(labels, 1, axis=1)                               # [B, T]
    in_maps = []
    for c in range(NCORES):
        s = slice(c * BL, (c + 1) * BL)
        xt = np.ascontiguousarray(X[s].T).astype(bff)                  # [IN, 32]
        lab = np.ascontiguousarray(shifted[s].T.reshape(TB, 1)).astype(np.int32)
        in_maps.append({**shared, "xt": xt, "lab": lab})
    return in_maps


def unpack_logits(raw: np.ndarray) -> np.ndarray:
    """[VS, NCH, 128, 512] bf16 -> [BL, T, V] fp32."""
    arr = np.asarray(raw).astype(np.float32)         # [40, 3, 128, 512]
    flat = arr.transpose(1, 3, 0, 2).reshape(NCH * 512, VPAD)  # [tb', v]
    flat = flat[:TB, :V]                             # [1280, 5000]
    return np.ascontiguousarray(
        flat.reshape(T, BL, V).transpose(1, 0, 2))   # [32, 40, 5000]


def run(inputs: dict, debug: bool = False, trace: bool = False):
    nc = _get_nc(debug=debug)
    in_maps = host_prep(inputs)
    r = run_bass_kernel_spmd(nc, in_maps, core_ids=list(range(NCORES)), trace=trace)
    outs = [unpack_logits(r.results[c]["logits8"]) for c in range(NCORES)]
    out = np.concatenate(outs, axis=0)
    return out, r


def kernel(**inputs) -> np.ndarray:
    out, _ = run(inputs, debug=False, trace=False)
    return out


if __name__ == "__main__":
    rng = np.random.default_rng(0)
    fake = {
        "X": rng.standard_normal((B, IN)).astype(np.float32),
        "labels": rng.integers(0, V, size=(B, T)).astype(np.int64),
        "W_f": (rng.standard_normal((E, IN)) * 0.02).astype(np.float32),
        "b_f": np.zeros(E, np.float32),
        "emb": (rng.standard_normal((V + 1, E)) * 0.02).astype(np.float32),
        "W_ih": (rng.standard_normal((G4, 2 * E)) * 0.02).astype(np.float32),
        "W_hh": (rng.standard_normal((G4, H)) * 0.02).astype(np.float32),
        "b_ih": np.zeros(G4, np.float32),
        "b_hh": np.zeros(G4, np.float32),
        "W_out": (rng.standard_normal((V, H)) * 0.02).astype(np.float32),
        "b_out": np.zeros(V, np.float32),
    }
    out = kernel(**fake)
    print("out", out.shape, out.dtype, float(np.abs(out).max()))
